# revision 14
# baseline (speedup 1.0000x reference)
"""AttentionConv2d pooling kernel for 8 Trainium2 NeuronCores.

Math: the reference computes, per batch n:
    tok = x[n].reshape(D, L).T                      # [L, D]
    K   = tok @ k_w.T + k_b + pos                   # [L, DOUT]
    V   = tok @ v_w.T + v_b                         # [L, DOUT]
    s   = K @ query / sqrt(DOUT)                    # [L]
    a   = softmax(s)                                # [L]
    out = a @ V                                     # [DOUT]

which collapses (since sum(a) == 1) to:
    q'  = k_w.T @ query / sqrt(DOUT)                # [D]
    ps  = (pos @ query + k_b @ query) / sqrt(DOUT)  # [L]   (fourier MLP)
    s   = x[n].T @ q' + ps                          # [L]
    u   = exp(s)        (scores are O(5), no max-subtraction needed)
    w   = x[n] @ u / sum(u)                         # [D]
    out = w @ v_w.T + v_b                           # [DOUT]

Sharding: data-parallel over batch N (2 batches per core); the fourier-MLP
pos-score is sharded over L across the 8 cores and AllGathered.
"""

import contextlib
import ctypes
import sys
import types

import numpy as np

# ---------------------------------------------------------------------------
# antenv.axon_hooks shim: the image lacks this module; bass_utils imports it
# to capture NTFF profiles when trace=True. Provide the ctypes equivalent.
# ---------------------------------------------------------------------------
if "antenv.axon_hooks" not in sys.modules:
    _HOOK_CACHE = []

    def _make_ntff_hook():
        try:
            lib = ctypes.CDLL("/opt/axon/libaxon_pjrt.so")
        except OSError:
            return None
        if not hasattr(lib, "axon_start_nrt_profile"):
            return None
        lib.axon_start_nrt_profile.argtypes = [
            ctypes.POINTER(ctypes.c_int64),
            ctypes.c_size_t,
        ]
        lib.axon_start_nrt_profile.restype = ctypes.c_int64
        lib.axon_stop_nrt_profile.argtypes = [ctypes.c_char_p]
        lib.axon_stop_nrt_profile.restype = ctypes.c_int64

        @contextlib.contextmanager
        def _hook(output_dir, device_ids):
            import jax

            jax.devices()
            if device_ids:
                ids = (ctypes.c_int64 * len(device_ids))(*device_ids)
                rc = lib.axon_start_nrt_profile(ids, len(device_ids))
            else:
                rc = lib.axon_start_nrt_profile(None, 0)
            if rc != 0:
                raise RuntimeError(f"axon_start_nrt_profile rc={rc}")
            try:
                yield
            finally:
                n = lib.axon_stop_nrt_profile(str(output_dir).encode())
                print(f"ntff profile: {n} file(s) written to {output_dir}")

        return _hook

    def get_axon_ntff_profile_hook():
        if not _HOOK_CACHE:
            _HOOK_CACHE.append(_make_ntff_hook())
        return _HOOK_CACHE[0]

    _mod = types.ModuleType("antenv.axon_hooks")
    _mod.get_axon_ntff_profile_hook = get_axon_ntff_profile_hook
    sys.modules["antenv.axon_hooks"] = _mod

import concourse.bass as bass  # noqa: E402
import concourse.mybir as mybir  # noqa: E402
import concourse.tile as tile  # noqa: E402
from concourse import bacc  # noqa: E402
from concourse.bass_utils import run_bass_kernel_spmd  # noqa: E402
from concourse.masks import make_identity  # noqa: E402

# Problem shapes (hardcoded per spec).
N, D, H, W = 16, 256, 128, 128
L = H * W  # 16384
DOUT = 256
NCORES = 8
NB = N // NCORES  # batches per core = 2
LSH = L // NCORES  # pos-score shard per core = 2048
LC = 2048  # l-chunk for the main loop
NSUB = LC // 512  # 512-column matmul subtiles per chunk
NCHUNK = L // LC  # chunks per batch = 8

F32 = mybir.dt.float32
F32R = mybir.dt.float32r
AF = mybir.ActivationFunctionType
OP = mybir.AluOpType

INV_SQRT_D = 1.0 / 16.0  # 1/sqrt(DOUT)
HALF_PI = float(np.pi / 2.0)
NLOC = 3  # leading l-ranges computed locally to bridge AllGather latency


def _r(ap):
    """Bitcast an fp32 AP to fp32r (fp22-truncated full-rate PE matmuls)."""
    return ap.bitcast(F32R)


def build_program(do_pre=True, do_cc=True, do_main=True, do_fin=True):
    nc = bacc.Bacc(
        "TRN2",
        target_bir_lowering=False,
        debug=False,
        enable_asserts=True,
        num_devices=NCORES,
    )

    # Per-core DRAM I/O. x_sh is this core's batch shard; gg is this core's
    # [gy; gx] grid rows for its pos-score L-shard (pure function of H, W).
    x_d = nc.dram_tensor("x_sh", [NB, D, L], F32, kind="ExternalInput").ap()
    query_d = nc.dram_tensor("query", [DOUT], F32, kind="ExternalInput").ap()
    kw_d = nc.dram_tensor("k_w", [DOUT, D], F32, kind="ExternalInput").ap()
    kb_d = nc.dram_tensor("k_b", [DOUT], F32, kind="ExternalInput").ap()
    vw_d = nc.dram_tensor("v_w", [DOUT, D], F32, kind="ExternalInput").ap()
    vb_d = nc.dram_tensor("v_b", [DOUT], F32, kind="ExternalInput").ap()
    wr_d = nc.dram_tensor("Wr", [DOUT // 2, 2], F32, kind="ExternalInput").ap()
    w1_d = nc.dram_tensor("w1", [DOUT, DOUT], F32, kind="ExternalInput").ap()
    b1_d = nc.dram_tensor("b1", [DOUT], F32, kind="ExternalInput").ap()
    w2_d = nc.dram_tensor("w2", [DOUT, DOUT], F32, kind="ExternalInput").ap()
    b2_d = nc.dram_tensor("b2", [DOUT], F32, kind="ExternalInput").ap()
    gg_d = nc.dram_tensor("gg", [2, 1 + NLOC, LSH], F32, kind="ExternalInput").ap()
    out_d = nc.dram_tensor("out", [NB, DOUT], F32, kind="ExternalOutput").ap()

    # Collective bounce buffers (internal DRAM; output must be Shared).
    pos_in_d = nc.dram_tensor("pos_in", [1, LSH], F32).ap()
    pos_gather_d = nc.dram_tensor(
        "pos_gather", [1, L], F32, addr_space="Shared"
    ).ap()

    with tile.TileContext(nc) as tc:
        with (
            tc.tile_pool(name="const", bufs=1) as cpool,
            tc.tile_pool(name="state", bufs=1) as spool,
        ):
            # live for the whole kernel
            q_rep = cpool.tile([128, 2, 128], F32R)  # q' replicated along free
            ones_row = cpool.tile([1, 128], F32R)
            vwT_sb = cpool.tile([128, 2, DOUT], F32)  # [d%128, d//128, o]
            vb_sb = cpool.tile([128, 2], F32)
            sexp_sb = spool.tile([128, NB * NCHUNK], F32)  # sum(exp) per chunk
            wpart_sb = spool.tile([128, 2, NB * NCHUNK], F32)  # [d%128, dh, idx]
            posall_sb = spool.tile([1, L], F32)  # gathered pos scores
            # locally computed pos for l-ranges 0..NLOC-1 (bridges the
            # AllGather latency: the first NLOC chunks don't wait for it)
            pos_loc_sb = spool.tile([1, NLOC * LSH], F32)

            with tc.tile_pool(name="pre", bufs=1) as ppool:
                # ---- constant loads (pos chain first) ------------------------
                gg_sb = ppool.tile([2, 1 + NLOC, LSH], F32)  # [gy;gx] x ranges
                nc.sync.dma_start(_r(gg_sb[:]), _r(gg_d))
                wrT_sb = ppool.tile([2, 128], F32)  # [k, f]
                nc.sync.dma_start(_r(wrT_sb[:]), _r(wr_d.rearrange("f k -> k f")))
                w1_sb = ppool.tile([128, 2, DOUT], F32)  # [j%128, j//128, f]
                nc.sync.dma_start(
                    w1_sb[:], w1_d.rearrange("(jh p) f -> p jh f", p=128)
                )
                b1_sb = ppool.tile([128, 2], F32)
                nc.sync.dma_start(b1_sb[:], b1_d.rearrange("(jh p) -> p jh", p=128))
                q_sb = ppool.tile([128, 2], F32)  # query as columns
                nc.sync.dma_start(q_sb[:], query_d.rearrange("(oh p) -> p oh", p=128))
                w2_sb = ppool.tile([128, 2, DOUT], F32)  # [o%128, o//128, j]
                nc.sync.dma_start(
                    w2_sb[:], w2_d.rearrange("(oh p) j -> p oh j", p=128)
                )
                kb_sb = ppool.tile([128, 2], F32)
                nc.sync.dma_start(kb_sb[:], kb_d.rearrange("(oh p) -> p oh", p=128))
                b2_sb = ppool.tile([128, 2], F32)
                nc.sync.dma_start(b2_sb[:], b2_d.rearrange("(oh p) -> p oh", p=128))
                kw_sb = ppool.tile([128, 2, D], F32)  # [o%128, o//128, d]
                nc.sync.dma_start(
                    kw_sb[:], kw_d.rearrange("(oh p) d -> p oh d", p=128)
                )
                vw_sb = ppool.tile([128, 2, D], F32)  # [o%128, o//128, d]
                nc.sync.dma_start(
                    vw_sb[:], vw_d.rearrange("(oh p) d -> p oh d", p=128)
                )
                nc.sync.dma_start(vb_sb[:], vb_d.rearrange("(oh p) -> p oh", p=128))

                ident_sb = ppool.tile([128, 128], F32)
                make_identity(nc, ident_sb[:])
                ones_tile = ppool.tile([128, 128], F32)
                nc.vector.memset(ones_tile[:], 1.0)
                halfpi_sb = ppool.tile([128, 1], F32)
                nc.vector.memset(halfpi_sb[:], HALF_PI)

                qs_sb = ppool.tile([128, 2], F32)  # query / sqrt(DOUT)
                nc.scalar.mul(qs_sb[:], q_sb[:], INV_SQRT_D)
                nc.scalar.mul(ones_row[:], ones_tile[0:1, :], 1.0)

                w1T_sb = ppool.tile([128, 2, DOUT], F32R)  # [f, fh, j] * 1/16
                pos_sh_sb = ppool.tile([1, LSH], F32)
                w2q_col = ppool.tile([128, 2], F32R)
                kb2_sb = ppool.tile([128, 2], F32)
                c_sb = ppool.tile([1, 1], F32)
                dummy_sb = ppool.tile([1, 1], F32)

                # ---- small matmuls + transposes ------------------------------
                nc.vector.tensor_tensor(
                    out=kb2_sb[:], in0=kb_sb[:], in1=b2_sb[:], op=OP.add
                )
                with (
                    tc.tile_pool(name="psT", bufs=2, space="PSUM") as psT,
                    tc.tile_pool(name="psA", bufs=2, space="PSUM") as psA,
                ):
                    for ah in range(2):
                        for bh in range(2):
                            ps_t = psT.tile([128, 128], F32, tag="tr")
                            nc.tensor.transpose(
                                ps_t[:],
                                w1_sb[:, ah, bh * 128 : (bh + 1) * 128],
                                ident_sb[:],
                            )
                            nc.vector.tensor_scalar_mul(
                                w1T_sb[:, bh, ah * 128 : (ah + 1) * 128],
                                ps_t[:],
                                INV_SQRT_D,
                            )
                    for jh in range(2):
                        ps_q = psA.tile([128, 1], F32, tag="vec")
                        for oh in range(2):
                            nc.tensor.matmul(
                                ps_q[:],
                                w2_sb[:, oh, jh * 128 : (jh + 1) * 128],
                                qs_sb[:, oh : oh + 1],
                                start=(oh == 0),
                                stop=(oh == 1),
                            )
                        nc.vector.tensor_copy(w2q_col[:, jh : jh + 1], ps_q[:])
                    ps_c = psA.tile([1, 1], F32, tag="sc")
                    for oh in range(2):
                        nc.tensor.matmul(
                            ps_c[:],
                            kb2_sb[:, oh : oh + 1],
                            qs_sb[:, oh : oh + 1],
                            start=(oh == 0),
                            stop=(oh == 1),
                        )
                    nc.vector.tensor_copy(c_sb[:], ps_c[:])
                    for dh in range(2):
                        ps_q = psA.tile([128, 1], F32, tag="vec")
                        for oh in range(2):
                            nc.tensor.matmul(
                                ps_q[:],
                                kw_sb[:, oh, dh * 128 : (dh + 1) * 128],
                                qs_sb[:, oh : oh + 1],
                                start=(oh == 0),
                                stop=(oh == 1),
                            )
                        qcol = ppool.tile([128, 1], F32, tag="qcol")
                        nc.vector.tensor_copy(qcol[:], ps_q[:])
                        nc.vector.tensor_scalar_mul(
                            q_rep[:, dh, :], ones_tile[:], qcol[:]
                        )
                    for ah in range(2):
                        for bh in range(2):
                            ps_t2 = psT.tile([128, 128], F32, tag="tr")
                            nc.tensor.transpose(
                                ps_t2[:],
                                vw_sb[:, ah, bh * 128 : (bh + 1) * 128],
                                ident_sb[:],
                            )
                            nc.vector.tensor_copy(
                                vwT_sb[:, bh, ah * 128 : (ah + 1) * 128], ps_t2[:]
                            )

                # ---- fourier-MLP pos scores: own shard first (feeds the
                # ---- AllGather), then NLOC leading ranges locally ------------
                last_hT = None
                with tc.tile_pool(name="psR", bufs=1, space="PSUM") as psR:
                    for r in range(1 + NLOC):
                        ps_rb = psR.tile([128, LSH], F32, tag="rb")
                        for s in range(NSUB):
                            sl = slice(s * 512, (s + 1) * 512)
                            nc.tensor.matmul(
                                ps_rb[:, sl], _r(wrT_sb[:]), _r(gg_sb[:, r, sl]),
                                start=True, stop=True,
                            )
                        cos_sb = ppool.tile([128, LSH], F32R, tag="cos")
                        sin_sb = ppool.tile([128, LSH], F32R, tag="sin")
                        nc.scalar.activation(
                            cos_sb[:], ps_rb[:], AF.Sin, bias=halfpi_sb[:]
                        )
                        nc.scalar.activation(sin_sb[:], ps_rb[:], AF.Sin)

                        hT_sb = ppool.tile([128, 2, LSH], F32R, tag="hT")
                        last_hT = hT_sb
                        for jh in range(2):
                            ps_h = psR.tile([128, LSH], F32, tag="rb")
                            for s in range(NSUB):
                                sl = slice(s * 512, (s + 1) * 512)
                                nc.tensor.matmul(
                                    ps_h[:, sl],
                                    w1T_sb[:, 0, jh * 128 : (jh + 1) * 128],
                                    cos_sb[:, sl],
                                    start=True, stop=False,
                                )
                                nc.tensor.matmul(
                                    ps_h[:, sl],
                                    w1T_sb[:, 1, jh * 128 : (jh + 1) * 128],
                                    sin_sb[:, sl],
                                    start=False, stop=True,
                                )
                            nc.scalar.activation(
                                hT_sb[:, jh, :], ps_h[:], AF.Gelu_apprx_tanh,
                                bias=b1_sb[:, jh : jh + 1],
                            )

                        ps_pos = psR.tile([1, LSH], F32, tag="pos")
                        for s in range(NSUB):
                            sl = slice(s * 512, (s + 1) * 512)
                            for jh in range(2):
                                nc.tensor.matmul(
                                    ps_pos[:, sl],
                                    w2q_col[:, jh : jh + 1],
                                    hT_sb[:, jh, sl],
                                    start=(jh == 0),
                                    stop=(jh == 1),
                                )
                        dest = (
                            pos_sh_sb[:]
                            if r == 0
                            else _r(pos_loc_sb[0:1, (r - 1) * LSH : r * LSH])
                        )
                        nc.vector.tensor_scalar_add(dest, ps_pos[:], c_sb[0:1, 0:1])

                        if r == 0 and do_cc:
                            nc.sync.dma_start(pos_in_d, pos_sh_sb[:])
                            nc.gpsimd.collective_compute(
                                "AllGather",
                                OP.bypass,
                                replica_groups=[list(range(NCORES))],
                                ins=[pos_in_d],
                                outs=[pos_gather_d],
                            )
                            nc.sync.dma_start(_r(posall_sb[:]), _r(pos_gather_d))
                if not do_cc:
                    nc.vector.memset(posall_sb[:], 0.0)
                    nc.scalar.mul(_r(posall_sb[:]), posall_sb[:], 1.0)

                # force the Exp table set to load now, not at the first
                # main-loop exp
                nc.scalar.activation(dummy_sb[:], last_hT[0:1, 0, 0:1], AF.Exp)

            # ---- main loop over (chunk, batch) ---------------------------
            with (
                tc.tile_pool(name="xp", bufs=4) as xpool,
                tc.tile_pool(name="up", bufs=2) as upool,
                tc.tile_pool(name="scr", bufs=1) as scrpool,
                tc.tile_pool(name="psM", bufs=2, space="PSUM") as psM,
            ):
                if not do_main:
                    nc.vector.memset(sexp_sb[:], 1.0)
                    nc.vector.memset(wpart_sb[:], 1.0)
                for c8 in range(NCHUNK if do_main else 0):
                    for n in range(NB):
                        idx = n * NCHUNK + c8
                        x_n = x_d[n].rearrange("(dh p) l -> p dh l", p=128)
                        x_t = xpool.tile([128, 2, LC], F32, tag="x")
                        nc.sync.dma_start(
                            _r(x_t[:]), _r(x_n[:, :, c8 * LC : (c8 + 1) * LC])
                        )

                        ps = psM.tile([128, LC], F32, tag="s")
                        for dh in range(2):
                            for s in range(NSUB):
                                sl = slice(s * 512, (s + 1) * 512)
                                nc.tensor.matmul(
                                    ps[:, sl],
                                    q_rep[:, dh, :],
                                    _r(x_t[:, dh, sl]),
                                    start=(dh == 0),
                                    stop=False,
                                )
                        pos_src = pos_loc_sb if c8 < NLOC else posall_sb
                        for s in range(NSUB):
                            lo = c8 * LC + s * 512
                            nc.tensor.matmul(
                                ps[:, s * 512 : (s + 1) * 512],
                                ones_row[:],
                                _r(pos_src[0:1, lo : lo + 512]),
                                start=False,
                                stop=True,
                            )

                        u_t = upool.tile([128, LC], F32, tag="u")
                        nc.scalar.activation(
                            u_t[:], ps[:], AF.Exp,
                            accum_out=sexp_sb[:, idx : idx + 1],
                        )

                        for dh in range(2):
                            scr = scrpool.tile([128, LC], F32, tag="scr")
                            nc.vector.affine_mul_reduce(
                                out=scr[:],
                                accum_out=wpart_sb[:, dh, idx : idx + 1],
                                in0=x_t[:, dh, :],
                                in1=u_t[:],
                                scale=1.0,
                                bias=0.0,
                            )

            # ---- normalize + V projection + store ------------------------
            with tc.tile_pool(name="fin", bufs=2) as fpool, tc.tile_pool(
                name="psF", bufs=2, space="PSUM"
            ) as psF:
                for n in range(NB if do_fin else 0):
                    csl = slice(n * NCHUNK, (n + 1) * NCHUNK)
                    s_col = fpool.tile([128, 1], F32, tag="scol")
                    nc.vector.tensor_reduce(
                        s_col[:], sexp_sb[:, csl], mybir.AxisListType.X, OP.add
                    )
                    srec = fpool.tile([128, 1], F32, tag="srec")
                    nc.vector.reciprocal(srec[:], s_col[:])

                    wn = fpool.tile([128, 2], F32, tag="wn")
                    for dh in range(2):
                        wsum = fpool.tile([128, 1], F32, tag="wsum")
                        nc.vector.tensor_reduce(
                            wsum[:], wpart_sb[:, dh, csl],
                            mybir.AxisListType.X, OP.add,
                        )
                        nc.vector.tensor_scalar_mul(
                            wn[:, dh : dh + 1], wsum[:], srec[:]
                        )

                    for oh in range(2):
                        ps_o = psF.tile([128, 1], F32, tag="o")
                        for dh in range(2):
                            nc.tensor.matmul(
                                ps_o[:],
                                vwT_sb[:, dh, oh * 128 : (oh + 1) * 128],
                                wn[:, dh : dh + 1],
                                start=(dh == 0),
                                stop=(dh == 1),
                            )
                        o_sb = fpool.tile([128, 1], F32, tag="osb")
                        nc.scalar.activation(
                            o_sb[:], ps_o[:], AF.Identity,
                            bias=vb_sb[:, oh : oh + 1],
                        )
                        nc.sync.dma_start(
                            out_d[n : n + 1, oh * 128 : (oh + 1) * 128], o_sb[:]
                        )

    nc.compile()
    return nc


_NC_CACHE = []


def _get_nc():
    if not _NC_CACHE:
        _NC_CACHE.append(build_program())
    return _NC_CACHE[0]


def _grid_rows():
    """[gy; gx] rows of the normalized meshgrid, flattened to length L."""
    ys = np.linspace(-1.0, 1.0, H, dtype=np.float64)
    xs = np.linspace(-1.0, 1.0, W, dtype=np.float64)
    gy = np.repeat(ys, W)
    gx = np.tile(xs, H)
    return np.stack([gy, gx]).astype(np.float32)  # [2, L]


def make_in_maps(inputs):
    x = np.ascontiguousarray(inputs["x"], dtype=np.float32).reshape(N, D, L)
    gg = _grid_rows()
    small = {
        k: np.ascontiguousarray(np.asarray(inputs[k], dtype=np.float32))
        for k in ("query", "k_w", "k_b", "v_w", "v_b", "Wr", "w1", "b1", "w2", "b2")
    }
    in_maps = []
    for c in range(NCORES):
        m = dict(small)
        m["x_sh"] = np.ascontiguousarray(x[c * NB : (c + 1) * NB])
        ranges = [c] + list(range(NLOC))
        ggc = np.stack(
            [gg[:, r * LSH : (r + 1) * LSH] for r in ranges], axis=1
        )  # [2, 1+NLOC, LSH]
        m["gg"] = np.ascontiguousarray(ggc)
        in_maps.append(m)
    return in_maps


def run(inputs, trace=False):
    nc = _get_nc()
    res = run_bass_kernel_spmd(
        nc, make_in_maps(inputs), core_ids=list(range(NCORES)), trace=trace
    )
    out = np.concatenate([res.results[c]["out"] for c in range(NCORES)], axis=0)
    return out.astype(np.float32), res


def kernel(**inputs) -> np.ndarray:
    out, _ = run(inputs, trace=False)
    return out
